# revision 19
# baseline (speedup 1.0000x reference)
"""AnalyticGaussianVelocity (soft-kNN flow velocity) on 8 trn2 NeuronCores.

Math (reference):
    a = t, b = 1-t
    logit[b,n] = -1/(2 b^2) * ||x_b - a * d_n||^2
    prob = softmax(logit, axis=n) * (1 + a/b)
    v = (-1/b) x + prob @ dataset

Dropping per-row constants, softmax(logit) == softmax(u * P) with
    u = a/b^2  (>0),  P[b,n] = x_b . d_n - (a/2) ||d_n||^2

Kernel strategy (v2):
  - dataset sharded over N across 8 cores; per-core flash-style online
    softmax; partial (m, l, acc) returned per core and merged on HOST
    (no collectives on device -> no cross-core sync/skew sensitivity).
  - all layout work on host: dataset pre-transposed and pre-split into
    f32r-exact 11-bit hi/lo components (f32r matmul is exact for 11-bit
    mantissas and runs 1 cyc/row like bf16; measured on HW via
    exp_f32r.py); norms and -(a/2) rows pre-split 2-way 11-bit so one
    K=4 matmul adds the norm term exactly.
  - rows sorted by t on host so precision tiers align with b-tiles:
    logit abs-error tolerance is ~0.1/u with u = a/(1-a)^2. Per-b-tile
    MM1 mode chosen from the tile's max u:
      "b" (u<=3):   one bf16 x bf16 pass        (noise ~0.036, cheap LDW)
      "f" (u<=33):  one f32r hi*hi pass         (noise ~3e-3; f32r pays a
                    serial ~53ns weight load per matmul - no FWL)
      "3" (rest):   f32r hi*hi + TWO BF16 correction passes hi*lo/lo*hi
                    (corrections only need ~8-bit components; ~2e-4)
  - MM2: prob (bf16 from ACT exp, row-sum free via accum_out) -> PE bf16
    transposes -> probT @ dataset_bf16; accumulator update
    acc = alpha*acc + pA fused on DVE reading PSUM directly (no
    diag-rescale matmul, no ACT acc copy). PE tail of b-tile i is
    emitted one b-tile late so PE never stalls on the softmax chain.
  - per n-tile, each operand class arrives as ONE coalesced DMA.

Measured (test.py, interleaved reps-differencing): ~650 us/core loop,
rel err 2.8e-3 (gate 2e-2). TimelineSim engine model: PE ~92% busy.
"""

import sys

sys.path.insert(0, "/opt/trn_rl_repo")

import numpy as np
import ml_dtypes

import concourse.bass as bass
import concourse.mybir as mybir
import concourse.tile as tile
from concourse import bacc
from concourse.bass_utils import run_bass_kernel_spmd
from concourse.masks import make_identity

B, D = 1024, 512
NCORES = 8
NTILE = 512  # dataset rows per n-tile
NBT = B // 128  # 8 b-tiles

F32 = mybir.dt.float32
F32R = mybir.dt.float32r
BF16 = mybir.dt.bfloat16

AF = mybir.ActivationFunctionType
OP = mybir.AluOpType
AX = mybir.AxisListType

# mode thresholds on per-b-tile max u (logit abs-error budget ~0.1):
# "b": single bf16xbf16 pass (noise ~0.036)   for u <= U_BF16
# "f": single f32r hi*hi pass (noise ~3e-3)   for u <= U_1PASS
# "3": f32r hi*hi + bf16 correction passes (noise ~2e-4) otherwise
U_BF16 = 3.0  # bf16 1-pass tiles measure ~11us faster than all-f32r (652 vs 665us)
U_1PASS = 33.0
USE_XBAR = False  # xbar transpose: confirmed racy at depth + slow (rel err 0.21, 1560us)


def build(n_tiles, tiers, reps=1, ndev=NCORES, skip=()):
    """tiers: NBT ints in {1,3} — MM1 passes per sorted b-tile.
    skip: subset of {"stats","tail","mm1"} for timing-attribution builds."""
    n_sh = n_tiles * NTILE
    nc = bacc.Bacc("TRN2", target_bir_lowering=False, debug=False, num_devices=ndev)

    # --- dram params (per core) ---
    # transposed dataset hi/lo, layout [t, k(128-chunk of d), 128 d, NTILE n]
    dsth_p = nc.declare_dram_parameter("dsth", [n_tiles * 128, 4 * NTILE], F32R, isOutput=False)
    dsthb_p = nc.declare_dram_parameter("dsthb", [n_tiles * 128, 4 * NTILE], BF16, isOutput=False)
    dstlb_p = nc.declare_dram_parameter("dstlb", [n_tiles * 128, 4 * NTILE], BF16, isOutput=False)
    # row-major bf16 dataset for MM2, layout [t, j(128-chunk of n), 128 n, D]
    natb_p = nc.declare_dram_parameter("natb", [n_tiles * 128, 4 * D], BF16, isOutput=False)
    # norm rows (wh.dnh + wh.dnl + wl.dnh + wl.dnl): dn4 = (dnh,dnl,dnh,dnl)
    dn4_p = nc.declare_dram_parameter("dn4", [4, n_sh], F32R, isOutput=False)
    # w4 = (wh,wh,wl,wl) where w = -(a/2) per sorted b row, [4, B]
    w4_p = nc.declare_dram_parameter("w4", [4, B], F32R, isOutput=False)
    # x^T hi/lo splits, layout [k(128-chunk of d), 128 d, B]
    xh_p = nc.declare_dram_parameter("xh", [4 * 128, B], F32R, isOutput=False)
    xb_p = nc.declare_dram_parameter("xb", [4 * 128, B], BF16, isOutput=False)
    xlb_p = nc.declare_dram_parameter("xlb", [4 * 128, B], BF16, isOutput=False)
    # per-b coefficient columns [128, NBT]: col i holds b = i*128+p (sorted order)
    ucol_p = nc.declare_dram_parameter("ucol", [128, NBT], F32, isOutput=False)
    nucol_p = nc.declare_dram_parameter("nucol", [128, NBT], F32, isOutput=False)
    # outputs: per-core partial softmax state
    m_out = nc.declare_dram_parameter("m_out", [128, NBT], F32, isOutput=True)
    l_out = nc.declare_dram_parameter("l_out", [128, NBT], F32, isOutput=True)
    acc_out = nc.declare_dram_parameter("acc_out", [NBT * 128, D], F32, isOutput=True)

    dsth_t = dsth_p.ap().rearrange("(t p) m -> t p m", p=128)
    dsthb_t = dsthb_p.ap().rearrange("(t p) m -> t p m", p=128)
    dstlb_t = dstlb_p.ap().rearrange("(t p) m -> t p m", p=128)
    natb_t = natb_p.ap().rearrange("(t p) m -> t p m", p=128)
    xh_t = xh_p.ap().rearrange("(k p) b -> k p b", p=128)
    xb_t = xb_p.ap().rearrange("(k p) b -> k p b", p=128)
    xlb_t = xlb_p.ap().rearrange("(k p) b -> k p b", p=128)
    acc_out_t = acc_out.ap().rearrange("(i p) d -> i p d", p=128)

    with tile.TileContext(nc) as tc:
        with (
            tc.tile_pool(name="persist", bufs=1) as pp,
            tc.tile_pool(name="dt", bufs=3) as dtp,
            tc.tile_pool(name="nat", bufs=3) as natp,
            tc.tile_pool(name="sf", bufs=4) as sfp,
            tc.tile_pool(name="tiny", bufs=6) as tp,
            tc.tile_pool(name="psL", bufs=4, space="PSUM") as psL,
            tc.tile_pool(name="psA", bufs=2, space="PSUM") as psA,
            tc.tile_pool(name="psT", bufs=2, space="PSUM") as psT,
        ):
            # ---------------- resident setup ----------------
            ident = pp.tile([128, 128], F32)
            make_identity(nc, ident[:])
            ident_bf = pp.tile([128, 128], BF16)
            nc.vector.tensor_copy(ident_bf[:], ident[:])

            xh_s = [pp.tile([128, B], F32R, name=f"xh{k}") for k in range(4)]
            xb_s = [pp.tile([128, B], BF16, name=f"xb{k}") for k in range(4)]
            xlb_s = [pp.tile([128, B], BF16, name=f"xlb{k}") for k in range(4)]
            for k in range(4):
                nc.sync.dma_start(out=xh_s[k][:], in_=xh_t[k])
                nc.sync.dma_start(out=xb_s[k][:], in_=xb_t[k])
            for k in range(4):
                nc.sync.dma_start(out=xlb_s[k][:], in_=xlb_t[k])
            w4_s = pp.tile([4, B], F32R)
            nc.sync.dma_start(out=w4_s[:], in_=w4_p.ap())
            dn4_s = pp.tile([4, n_sh], F32R)
            nc.sync.dma_start(out=dn4_s[:], in_=dn4_p.ap())
            ucol = pp.tile([128, NBT], F32)
            nucol = pp.tile([128, NBT], F32)
            nc.sync.dma_start(out=ucol[:], in_=ucol_p.ap())
            nc.sync.dma_start(out=nucol[:], in_=nucol_p.ap())

            m_run = pp.tile([128, NBT], F32)
            l_run = pp.tile([128, NBT], F32)
            acc = [pp.tile([128, D], F32, name=f"acc{i}") for i in range(NBT)]

            for _rep in range(reps):
                nc.vector.memset(m_run[:], -1.0e30)
                nc.vector.memset(l_run[:], 0.0)
                for i in range(NBT):
                    nc.vector.memset(acc[i][:], 0.0)
                pending = []

                # ---------------- main loop over dataset tiles ----------------
                for t in range(n_tiles):
                    dTha = dtp.tile([128, 4 * NTILE], F32R, tag="dTha", name="dTha")
                    dThba = dtp.tile([128, 4 * NTILE], BF16, tag="dThba", name="dThba")
                    dTlba = dtp.tile([128, 4 * NTILE], BF16, tag="dTlba", name="dTlba")
                    nba = natp.tile([128, 4 * D], BF16, tag="nba", name="nba")
                    nc.sync.dma_start(out=dTha[:], in_=dsth_t[t])
                    nc.sync.dma_start(out=dThba[:], in_=dsthb_t[t])
                    nc.sync.dma_start(out=dTlba[:], in_=dstlb_t[t])
                    nc.sync.dma_start(out=nba[:], in_=natb_t[t])
                    dTh = [dTha[:, k * NTILE:(k + 1) * NTILE] for k in range(4)]
                    dThb = [dThba[:, k * NTILE:(k + 1) * NTILE] for k in range(4)]
                    dTlb = [dTlba[:, k * NTILE:(k + 1) * NTILE] for k in range(4)]
                    nb = [nba[:, j * D:(j + 1) * D] for j in range(4)]
                    sl_n = slice(t * NTILE, (t + 1) * NTILE)

                    def emit_tail(i, prob, alpha, nb):
                        """PE tail of b-tile i: probT transposes + MM2, then
                        the DVE accumulator update. Emitted one b-tile late
                        so PE never waits on the softmax chain."""
                        probT = sfp.tile([128, NTILE], BF16, tag="probT", name="probT")
                        if USE_XBAR:
                            for k in range(4):
                                ksl = slice(k * 128, (k + 1) * 128)
                                nc.sync.dma_start_transpose(probT[:, ksl], prob[:, ksl])
                        else:
                            pP = psT.tile([128, NTILE], BF16, tag="pP", name="pP")
                            for k in range(4):
                                ksl = slice(k * 128, (k + 1) * 128)
                                nc.tensor.transpose(pP[:, ksl], prob[:, ksl], ident_bf[:])
                            nc.scalar.copy(probT[:], pP[:])
                        pA = psA.tile([128, D], F32, tag="pA", name="pA")
                        for k in range(4):
                            ksl = slice(k * 128, (k + 1) * 128)
                            nc.tensor.matmul(
                                pA[:], probT[:, ksl], nb[k],
                                start=(k == 0), stop=(k == 3),
                            )
                        # acc = alpha*acc + pA (fused DVE, reads PSUM)
                        nc.vector.scalar_tensor_tensor(
                            out=acc[i][:], in0=acc[i][:],
                            scalar=alpha[:], in1=pA[:], op0=OP.mult, op1=OP.add,
                        )

                    for i in range(NBT):
                        bi = slice(i * 128, (i + 1) * 128)
                        pL = psL.tile([128, NTILE], F32, tag="pL")
                        mode = tiers[i]
                        if mode == "b":
                            passes = ((xb_s, dThb),)
                        elif mode == "f":
                            passes = ((xh_s, dTh),)
                        else:
                            passes = ((xh_s, dTh), (xb_s, dTlb), (xlb_s, dThb))
                        if "mm1" in skip:
                            passes = ()
                        first = True
                        for hk, dk in passes:
                            for k in range(4):
                                nc.tensor.matmul(
                                    pL[:], hk[k][:, bi], dk[k],
                                    start=first, stop=False,
                                )
                                first = False
                        nc.tensor.matmul(
                            pL[:], w4_s[:, bi], dn4_s[:, sl_n], start=first, stop=True
                        )

                        if "stats" in skip:
                            continue
                        # online softmax stats
                        mt = tp.tile([128, 1], F32, tag="mt")
                        nc.vector.tensor_reduce(mt[:], pL[:], axis=AX.X, op=OP.max)
                        dlt = tp.tile([128, 1], F32, tag="dlt")
                        # dlt = min(m_old - mt, 0) = m_old - m_new
                        nc.vector.tensor_scalar(
                            out=dlt[:], in0=m_run[:, i:i + 1], scalar1=mt[:],
                            scalar2=0.0, op0=OP.subtract, op1=OP.min,
                        )
                        nc.vector.tensor_tensor(
                            m_run[:, i:i + 1], m_run[:, i:i + 1], mt[:], op=OP.max
                        )
                        alpha = tp.tile([128, 1], F32, tag="alpha")
                        nc.scalar.activation(
                            alpha[:], dlt[:], AF.Exp, bias=0.0, scale=ucol[:, i:i + 1]
                        )
                        ebias = tp.tile([128, 1], F32, tag="ebias")
                        nc.vector.tensor_tensor(
                            ebias[:], nucol[:, i:i + 1], m_run[:, i:i + 1], op=OP.mult
                        )
                        # prob = exp(u*P + bias) in bf16, lt = rowsum
                        prob = sfp.tile([128, NTILE], BF16, tag="prob")
                        lt = tp.tile([128, 1], F32, tag="lt")
                        nc.scalar.activation(
                            prob[:], pL[:], AF.Exp,
                            bias=ebias[:], scale=ucol[:, i:i + 1], accum_out=lt[:],
                        )
                        # l = l*alpha + lt (fused DVE)
                        nc.vector.scalar_tensor_tensor(
                            out=l_run[:, i:i + 1], in0=l_run[:, i:i + 1],
                            scalar=alpha[:], in1=lt[:], op0=OP.mult, op1=OP.add,
                        )
                        if "tail" in skip:
                            continue
                        # 2-deep PE software pipeline: the tail of b-tile i
                        # is emitted after MM1 of b-tiles i+1 and i+2
                        if len(pending) >= 2:
                            emit_tail(*pending.pop(0))
                        pending.append((i, prob, alpha, nb))

                for args_t in pending:
                    emit_tail(*args_t)

                # ---------------- write partial state ----------------
                nc.sync.dma_start(out=m_out.ap(), in_=m_run[:])
                nc.sync.dma_start(out=l_out.ap(), in_=l_run[:])
                for i in range(NBT):
                    nc.sync.dma_start(out=acc_out_t[i], in_=acc[i][:])

    nc.compile()
    return nc


_BUILD_CACHE = {}


def _get_nc(n_tiles, tiers, reps=1, ndev=NCORES, skip=()):
    key = (n_tiles, tuple(tiers), reps, ndev, tuple(skip), USE_XBAR)
    if key not in _BUILD_CACHE:
        _BUILD_CACHE[key] = build(n_tiles, tuple(tiers), reps=reps, ndev=ndev, skip=tuple(skip))
    return _BUILD_CACHE[key]


def _rne11(x):
    """Round fp32 to 11 explicit mantissa bits (f32r-exact), round-half-even."""
    xi = np.ascontiguousarray(x, dtype=np.float32).view(np.uint32)
    keep = np.uint32(0xFFFFF000)
    half = np.uint32(0x800)
    odd = (xi >> np.uint32(12)) & np.uint32(1)
    r = (xi + (half - np.uint32(1)) + odd) & keep
    return r.view(np.float32)


def _trunc11(x):
    xi = np.ascontiguousarray(x, dtype=np.float32).view(np.uint32)
    return (xi & np.uint32(0xFFFFF000)).view(np.float32)


def _split11(x):
    hi = _rne11(x)
    lo = _trunc11((x.astype(np.float32) - hi))
    return hi, lo


def prepare(x_t, t, dataset, n_tiles):
    """Host-side layout: sort rows by t, pad+shard dataset, pre-split."""
    bf16 = ml_dtypes.bfloat16
    n = dataset.shape[0]
    n_pad = NCORES * n_tiles * NTILE
    assert n_pad >= n

    perm = np.argsort(t, kind="stable")
    xs = np.ascontiguousarray(x_t[perm])
    ts = t[perm].astype(np.float64)

    a = ts
    b = 1.0 - a
    u = (a / (b * b)).astype(np.float32)
    w = (-a / 2.0).astype(np.float32)
    dcoef = (1.0 + a / b)
    vcoef = (-1.0 / b)

    # per-b-tile MM1 mode from max u in tile
    umax = u.reshape(NBT, 128).max(axis=1)
    tiers = tuple(
        "b" if um <= U_BF16 else ("f" if um <= U_1PASS else "3") for um in umax
    )

    # dataset: pad with far-away rows (value 0, huge norm)
    dn = np.einsum("nd,nd->n", dataset, dataset, dtype=np.float64).astype(np.float32)
    dn_pad = np.full(n_pad, 1.0e6, dtype=np.float32)
    dn_pad[:n] = dn
    dpad = np.zeros((n_pad, D), dtype=np.float32)
    dpad[:n] = dataset

    ds_hi32, ds_lo32 = _split11(dpad)
    # transposed layouts: [core, t, k, p(128 d), NTILE]
    def tsplit(z):
        # z: [n_pad, D] -> [core, t, p(128 d), k, NTILE]
        zt = z.T.reshape(4, 128, NCORES, n_tiles, NTILE)
        return np.ascontiguousarray(zt.transpose(2, 3, 1, 0, 4))

    dsth = tsplit(ds_hi32)
    dsthb = tsplit(dpad.astype(bf16))
    dstlb = tsplit(ds_lo32.astype(bf16))
    natb = np.ascontiguousarray(
        dpad.astype(bf16).reshape(NCORES, n_tiles, 4, 128, D).transpose(0, 1, 3, 2, 4)
    )

    dnh, dnl = _split11(dn_pad)
    dn4 = np.stack([dnh, dnl, dnh, dnl]).reshape(4, NCORES, n_tiles * NTILE)
    wh, wl = _split11(w)
    w4 = np.ascontiguousarray(np.stack([wh, wh, wl, wl]))

    xT = np.ascontiguousarray(xs.T).reshape(4, 128, B)
    xh, xl = _split11(xT)

    def col(v):
        return np.ascontiguousarray(v.astype(np.float32).reshape(NBT, 128).T)

    base = dict(
        xh=xh.reshape(4 * 128, B),
        xb=np.ascontiguousarray(xT.astype(bf16)).reshape(4 * 128, B),
        xlb=xl.astype(bf16).reshape(4 * 128, B),
        w4=w4,
        ucol=col(u),
        nucol=col(-u),
    )
    in_maps = [
        dict(
            base,
            dsth=dsth[c].reshape(n_tiles * 128, 4 * NTILE),
            dsthb=dsthb[c].reshape(n_tiles * 128, 4 * NTILE),
            dstlb=dstlb[c].reshape(n_tiles * 128, 4 * NTILE),
            natb=natb[c].reshape(n_tiles * 128, 4 * D),
            dn4=np.ascontiguousarray(dn4[:, c]),
        )
        for c in range(NCORES)
    ]
    aux = dict(perm=perm, u=u, dcoef=dcoef, vcoef=vcoef, xs=xs, tiers=tiers)
    return in_maps, aux


def merge(results, aux):
    """Host-side flash-softmax merge of per-core partials -> full output."""
    u = aux["u"].astype(np.float64)  # [B] sorted order
    # device layout: [128, NBT] col i holds b = i*128+p -> transpose+flatten
    def uncol(z):
        return np.asarray(z, dtype=np.float64).T.reshape(B)

    ms = np.stack([uncol(r["m_out"]) for r in results])  # [C, B]
    ls = np.stack([uncol(r["l_out"]) for r in results])  # [C, B]
    accs = np.stack([np.asarray(r["acc_out"], dtype=np.float64) for r in results])

    m_glob = ms.max(axis=0)  # [B]
    gam = np.exp(u[None, :] * (ms - m_glob[None, :]))  # [C, B]
    l_glob = (gam * ls).sum(axis=0)  # [B]
    acc_glob = np.einsum("cb,cbd->bd", gam, accs)  # [B, D]

    v_sorted = (
        aux["dcoef"][:, None] * acc_glob / l_glob[:, None]
        + aux["vcoef"][:, None] * aux["xs"].astype(np.float64)
    )
    v = np.empty((B, D), dtype=np.float32)
    v[aux["perm"]] = v_sorted.astype(np.float32)
    return v


def kernel(x_t, t, dataset):
    x_t = np.asarray(x_t, dtype=np.float32)
    t = np.asarray(t, dtype=np.float32)
    dataset = np.asarray(dataset, dtype=np.float32)
    n = dataset.shape[0]
    n_tiles = -(-n // (NCORES * NTILE))  # ceil -> 25 for N=100000
    in_maps, aux = prepare(x_t, t, dataset, n_tiles)
    nc = _get_nc(n_tiles, aux["tiers"])
    res = run_bass_kernel_spmd(nc, in_maps, core_ids=list(range(NCORES)))
    return merge(res.results, aux)


def ref_numpy(x_t, t, dataset):
    aa = t.astype(np.float64)
    bb = 1.0 - aa
    dsn = (dataset.astype(np.float64) ** 2).sum(1)
    t2 = x_t.astype(np.float64) @ dataset.T.astype(np.float64)
    logit = (-1.0 / (2 * bb * bb))[:, None] * (
        (x_t.astype(np.float64) ** 2).sum(1)[:, None]
        - 2 * aa[:, None] * t2
        + (aa * aa)[:, None] * dsn[None, :]
    )
    p = np.exp(logit - logit.max(1, keepdims=True))
    p /= p.sum(1, keepdims=True)
    p = p * (1 + aa / bb)[:, None]
    return (-1.0 / bb)[:, None] * x_t.astype(np.float64) + p @ dataset.astype(np.float64)


if __name__ == "__main__":
    rng = np.random.default_rng(0)
    n = 2 * NCORES * NTILE - 300
    x_t = rng.standard_normal((B, D)).astype(np.float32)
    t = rng.uniform(0.05, 0.95, (B,)).astype(np.float32)
    dataset = rng.standard_normal((n, D)).astype(np.float32)
    v = kernel(x_t, t, dataset)
    vref = ref_numpy(x_t, t, dataset)
    err = np.linalg.norm(v - vref) / np.linalg.norm(vref)
    print("rel l2 err:", err)
    print("max abs err:", np.abs(v - vref).max(), "ref scale:", np.abs(vref).max())
